# revision 12
# baseline (speedup 1.0000x reference)
"""Multi-head attention (B=4, N=2048, dim=768, H=16, d_k=48) on 8 TRN2 NeuronCores.

Sharding: data-parallel over (batch, query-half): core c handles batch c//2,
queries [1024*(c%2), 1024*(c%2+1)).  K/V are computed per-core for the full
batch element (replicated across the 2 cores sharing a batch), so there are
no collectives.

PE-tiling strategy (the main speedup over the per-head baseline):
  - Scores contract over d_k=48 only, so a single head uses 48/128 PE rows.
    Heads are processed in PAIRS: head A's K/Q live at partitions 0-47, head
    B's at 64-111 (pair-padded weights), and the two scores matmuls are
    issued back-to-back as 64x128 row-tiles (tile_position (0,0)/(64,0)) --
    the PE runs them CONCURRENTLY in disjoint row groups => ~2x.
  - AV produces only 49 output rows per head (48 dims + sums row), so a
    single head uses 49/128 PE output columns.  The pair's two AV matmuls
    are issued as 128x64 col-tiles (tile_position (0,0)/(0,64)): head A
    accumulates into PSUM partitions 0-48, head B into 64-112 of the SAME
    banks, concurrently => ~2x.
  - Projections (full 128x128) run as fillers between attention steps.
  - Softmax denominators fall out of AV as an extra ones-column in V-hat.
    Normalization per pair: ScalarE copies the two sum rows, DRAM-bounce to
    partition base 0, one DVE reciprocal, then a mask-matmul broadcasts the
    reciprocals to partitions 0-47 / 64-111 so the DVE multiplies are
    partition-aligned with psO.  The whole chain runs as a pumped generator
    (latency off the critical path).
"""

import numpy as np
import ml_dtypes

BF16 = ml_dtypes.bfloat16
DIM = 768
H = 16
DK = 48
B = 4
N = 2048
QH = 1024           # queries per core
NCORES = 8
KT = N // 128       # 16 key tiles
PAIRS = H // 2      # 8 head pairs (one padded 128-row weight tile each)
INV_SQRT_DK = 1.0 / float(np.sqrt(DK))
VPAD = 49          # V columns: 48 data + ones column at 48
SUMCOL = 48
# Schraudolph bf16: bits16 = round(s * SCH_A + SCH_B) reinterpreted as bf16
# approximates exp(s / sqrt(DK)); SCH_B folds the standard -0.0579 correction.
SCH_A = 128.0 * float(np.log2(np.e)) * INV_SQRT_DK
SCH_B = 127.0 * 128.0 - 7.4109

_compiled = None


def _emit(tc, nc):
    import concourse.mybir as mybir
    from concourse.bass import ts

    f32 = mybir.dt.float32
    bf16 = mybir.dt.bfloat16
    i16 = mybir.dt.int16
    f32r = mybir.dt.float32r
    Ident = mybir.ActivationFunctionType.Identity
    Exp = mybir.ActivationFunctionType.Exp

    def dp(name):
        return nc.dram_tensor_handles[name].ap()

    xT = dp("xT")
    wqT = dp("wqT")
    wkT = dp("wkT")
    wvT = dp("wvT")
    woT = dp("woT")
    qb = dp("qb")
    kb = dp("kb")
    out = dp("out")

    sync = nc.sync

    def _try_skip_ldw(mm_result):
        try:
            mm_result.ins.ldweights = False
        except Exception:
            pass

    persist = tc.alloc_tile_pool(name="persist", bufs=1)

    def single(name, shape, dtype):
        return persist.tile(shape, dtype, name=name, tag=name)

    # ---- persistent SBUF tensors ----
    XT = [single(f"XT{j}", [128, N], bf16) for j in range(6)]
    WQ = [single(f"WQ{j}", [128, PAIRS * 128], bf16) for j in range(6)]
    WK = [single(f"WK{j}", [128, PAIRS * 128], bf16) for j in range(6)]
    WV = [single(f"WV{j}", [128, DIM], bf16) for j in range(6)]
    WO = [single(f"WO{j}", [128, DIM], bf16) for j in range(6)]
    QT = [single(f"QT{p}", [128, QH], bf16) for p in range(PAIRS)]
    KTB = [single(f"KTB{p}", [128, N], bf16) for p in range(PAIRS)]
    VT = [single(f"VT{i}", [128, H, VPAD], bf16) for i in range(KT)]
    XA = [single(f"XA{j}", [128, QH], bf16) for j in range(6)]
    qb_sb = single("qb_sb", [128, PAIRS], f32)
    kb_sb = single("kb_sb", [128, PAIRS], f32)
    # broadcast mask for the norm: bmask[0, 0:48]=1, bmask[1, 64:112]=1
    bmask = single("bmask", [2, 128], bf16)
    bstage = single("bstage", [1, 256], bf16)

    rs_dram = [nc.dram_tensor(f"rsd{k}", [2, QH], f32).ap() for k in range(2)]

    psS = tc.alloc_tile_pool(name="psS", bufs=3, space="PSUM")   # 3 banks
    psO_pool = tc.alloc_tile_pool(name="psO", bufs=2, space="PSUM")  # 4 banks
    psM = tc.alloc_tile_pool(name="psM", bufs=1, space="PSUM")   # 1 bank
    ptp = tc.alloc_tile_pool(name="ptp", bufs=14)
    rsp = tc.alloc_tile_pool(name="rsp", bufs=2)
    xap = tc.alloc_tile_pool(name="xap", bufs=2)
    outp = tc.alloc_tile_pool(name="outp", bufs=3)

    # ---- input DMAs round-robined over 3 HWDGE rings ----
    loads = []
    for j in range(6):
        loads.append((WQ[j][:], wqT[ts(j, 128), :, :]))
        loads.append((XT[j][:], xT[ts(j, 128), :]))
    loads.append((qb_sb[:], qb[:, :]))
    loads.append((kb_sb[:], kb[:, :]))
    for j in range(6):
        loads.append((WK[j][:], wkT[ts(j, 128), :, :]))
        loads.append((WV[j][:], wvT[ts(j, 128), :]))
    for j in range(6):
        loads.append((WO[j][:], woT[ts(j, 128), :]))
    rings = [sync, nc.scalar, nc.gpsimd]
    for n, (dst, src) in enumerate(loads):
        rings[n % 3].dma_start(out=dst, in_=src)

    # ones column in V-hat; broadcast mask rows
    for i in range(KT):
        nc.vector.memset(VT[i][:, :, SUMCOL:VPAD], 1.0)
    nc.vector.memset(bstage[:], 0.0)
    nc.vector.memset(bstage[0:1, 0:DK], 1.0)
    nc.vector.memset(bstage[0:1, 128 + 64:128 + 64 + DK], 1.0)
    sync.dma_start(out=bmask[0:1, :], in_=bstage[0:1, 0:128])
    sync.dma_start(out=bmask[1:2, :], in_=bstage[0:1, 128:256])

    # ---- projection generators (fillers) ----
    # Each accumulation group targets a single PSUM bank ([128, <=512] f32)
    # from the shared psM pool so attention keeps 7 banks.
    def q_gen(p):
        for c in range(2):
            ps = psM.tile([128, 512], f32, name=f"psQ{p}_{c}", tag="PSM")
            for k in range(6):
                nc.tensor.matmul(
                    out=ps[:],
                    lhsT=WQ[k][:, ts(p, 128)],
                    rhs=XT[k][:, ts(c, 512)],
                    start=(k == 0), stop=(k == 5),
                )
                yield
            nc.vector.tensor_scalar(
                out=QT[p][:, ts(c, 512)], in0=ps[:],
                scalar1=qb_sb[:, p:p + 1], scalar2=None,
                op0=mybir.AluOpType.add,
            )
            yield

    def k_gen(p):
        for c in range(4):
            ps = psM.tile([128, 512], f32, name=f"psK{p}_{c}", tag="PSM")
            for k in range(6):
                nc.tensor.matmul(
                    out=ps[:],
                    lhsT=WK[k][:, ts(p, 128)],
                    rhs=XT[k][:, ts(c, 512)],
                    start=(k == 0), stop=(k == 5),
                )
                yield
            nc.vector.tensor_scalar(
                out=KTB[p][:, ts(c, 512)], in0=ps[:],
                scalar1=kb_sb[:, p:p + 1], scalar2=None,
                op0=mybir.AluOpType.add,
            )
            yield

    def v_gen(i):
        for cc, h0 in enumerate([0, 8]):
            ps = psM.tile([128, 384], f32, name=f"psV{i}_{cc}", tag="PSM")
            for k in range(6):
                nc.tensor.matmul(
                    out=ps[:],
                    lhsT=XT[k][:, ts(i, 128)],
                    rhs=WV[k][:, h0 * DK:h0 * DK + 384],
                    start=(k == 0), stop=(k == 5),
                )
                yield
            nc.vector.tensor_copy(
                VT[i][:, h0:h0 + 8, 0:DK],
                ps[:].rearrange("p (h d) -> p h d", h=8),
            )
            yield

    # ---- attention ----
    psO_of = {}
    pt_of = {}

    def scores_pair(p, kt):
        # 4 row-tiled matmuls; A (rows 0-47) and B (rows 64-111) run
        # concurrently in disjoint PE row groups.  Eviction split: ScalarE
        # takes 3 of 4 tiles (full exp), DVE takes (B, c0) via Schraudolph.
        for c in range(2):
            for hh in range(2):
                off = 64 * hh
                ps = psS.tile([128, 512], f32, name=f"psS{p}_{kt}_{c}_{hh}", tag="PSS")
                nc.tensor.matmul(
                    out=ps[:],
                    lhsT=KTB[p][off:off + DK, ts(kt, 128)],
                    rhs=QT[p][off:off + DK, ts(c, 512)],
                    start=True, stop=True,
                )
                pt = ptp.tile([128, 512], bf16, name=f"pt{p}_{kt}_{c}_{hh}", tag="PT")
                pt_of[(p, kt, c, hh)] = pt
                if hh == 1 and c == 0:
                    nc.vector.tensor_scalar(
                        out=pt[:].bitcast(i16), in0=ps[:],
                        scalar1=SCH_A, scalar2=SCH_B,
                        op0=mybir.AluOpType.mult, op1=mybir.AluOpType.add,
                    )
                else:
                    nc.scalar.activation(pt[:], ps[:], Exp, scale=INV_SQRT_DK)

    def av_pair(p, kt):
        # 4 col-tiled matmuls; head A -> PSUM partitions 0-48, head B ->
        # 64-112 of the same banks, concurrent in disjoint PE col groups.
        if kt == 0:
            psO_of[p] = psO_pool.tile([128, QH], f32, name=f"psO{p}", tag="PSO")
        psO = psO_of[p]
        for c in range(2):
            for hh in range(2):
                pt = pt_of.pop((p, kt, c, hh))
                nc.tensor.matmul(
                    out=psO[64 * hh:64 * hh + VPAD, ts(c, 512)],
                    lhsT=VT[kt][:, 2 * p + hh, 0:VPAD],
                    rhs=pt[:],
                    start=(kt == 0), stop=(kt == KT - 1),
                    tile_position=(0, 64 * hh),
                )

    def norm_gen(p):
        # pair normalization: sums rows -> DRAM bounce to base 0 -> one DVE
        # reciprocal -> mask-matmul broadcast to partitions 0-47/64-111 ->
        # partition-aligned DVE multiplies -> DMA scatter into XA.
        psO = psO_of.pop(p)
        srow = rsp.tile([128, QH], f32, name=f"srow{p}", tag="RS")
        # engine ops need 32-aligned partition bases; copy an aligned 32-row
        # superset, then DMA the exact sum row (DMAs are unrestricted)
        nc.scalar.activation(srow[32:64, :], psO[32:64, :], Ident, scale=1.0)
        nc.scalar.activation(srow[96:128, :], psO[96:128, :], Ident, scale=1.0)
        rsd = rs_dram[p % 2]
        sync.dma_start(out=rsd[0:1, :], in_=srow[SUMCOL:SUMCOL + 1, :])
        sync.dma_start(out=rsd[1:2, :], in_=srow[64 + SUMCOL:64 + SUMCOL + 1, :])
        yield
        den = rsp.tile([2, QH], f32, name=f"den{p}", tag="DEN")
        sync.dma_start(out=den[:], in_=rsd[:, :])
        yield
        nc.vector.reciprocal_approx_fast(out=den[:], in_=den[:])
        yield
        denb = rsp.tile([2, QH], bf16, name=f"denb{p}", tag="DENB")
        nc.scalar.activation(denb[:], den[:], Ident, scale=1.0)
        yield
        xa = xap.tile([128, QH], bf16, name=f"xa{p}", tag="XAH")
        for c in range(2):
            psBC = psM.tile([128, 512], f32, name=f"psBC{p}_{c}", tag="PSM")
            nc.tensor.matmul(
                out=psBC[0:113, :],
                lhsT=bmask[:, 0:113],
                rhs=denb[:, ts(c, 512)],
                start=True, stop=True,
            )
            yield
            rsb = rsp.tile([128, 512], f32, name=f"rsb{p}_{c}", tag="RSB")
            nc.scalar.activation(rsb[0:113, :], psBC[0:113, :], Ident, scale=1.0)
            yield
            nc.vector.tensor_mul(xa[0:DK, ts(c, 512)], psO[0:DK, ts(c, 512)], rsb[0:DK, :])
            nc.vector.tensor_mul(xa[64:64 + DK, ts(c, 512)], psO[64:64 + DK, ts(c, 512)], rsb[64:64 + DK, :])
            yield
        # scatter pair rows into f-major X_att^T tiles: head A rows 96p..+48,
        # head B rows 96p+48..+96 (each may straddle one 128-row tile edge)
        for hh in range(2):
            r = 96 * p + DK * hh
            f0, r0 = r // 128, r % 128
            n1 = min(128 - r0, DK)
            sync.dma_start(out=XA[f0][r0:r0 + n1, :], in_=xa[64 * hh:64 * hh + n1, :])
            if n1 < DK:
                sync.dma_start(out=XA[f0 + 1][0:DK - n1, :], in_=xa[64 * hh + n1:64 * hh + DK, :])
        yield

    # ---- out_proj tail ----
    o2_box = []

    def out_proj_04(t, pool, tg):
        # k-chunks 0..4 only touch XA[0..4] (pairs <= 6), so these run while
        # pair 7's normalization drains.  Output cols split 512+256 so each
        # accumulation group fits one PSUM bank.
        tiles = []
        for g, (base, w) in enumerate([(0, 512), (512, 256)]):
            ps = pool.tile([128, w], f32, name=f"psY{t}_{g}", tag=tg)
            for k in range(5):
                nc.tensor.matmul(
                    out=ps[:],
                    lhsT=XA[k][:, ts(t, 128)],
                    rhs=WO[k][:, base:base + w],
                    start=(k == 0), stop=False,
                )
            tiles.append(ps)
        return tiles

    def out_proj_5(t, tiles):
        for g, (base, w) in enumerate([(0, 512), (512, 256)]):
            nc.tensor.matmul(
                out=tiles[g][:],
                lhsT=XA[5][:, ts(t, 128)],
                rhs=WO[5][:, base:base + w],
                start=False, stop=True,
            )
        tt = t % 2
        if tt == 0:
            o2_box.append(outp.tile([128, 2, DIM], bf16, name=f"o2_{t // 2}", tag="OUT"))
        o2 = o2_box[-1]
        nc.scalar.activation(o2[:, tt, 0:512], tiles[0][:], Ident, scale=1.0)
        nc.scalar.activation(o2[:, tt, 512:DIM], tiles[1][:], Ident, scale=1.0)
        if tt == 1:
            (sync if (t // 2) % 2 == 0 else nc.scalar).dma_start(
                out=out[:, t - 1:t + 1, :], in_=o2[:, :, :]
            )

    # ---- schedule ----
    from collections import deque

    fillers = deque()

    def pump(n):
        done = 0
        while fillers and done < n:
            try:
                next(fillers[0])
                done += 1
            except StopIteration:
                fillers.popleft()

    for g in (q_gen(0), k_gen(0), v_gen(0)):
        for _ in g:
            pass

    for i in range(1, KT):
        fillers.append(v_gen(i))

    av_q = deque()
    LAG = 2  # in kt steps
    for p in range(PAIRS):
        if p + 1 < PAIRS:
            fillers.append(q_gen(p + 1))
            fillers.append(k_gen(p + 1))
        # pair 0 must drain all v_gen fillers fast enough that v_gen(kt)
        # lands before av_pair(0, kt) enters the PE stream (engine-order
        # hazard); later pairs just need the next pair's q/k + prev norm.
        budget = 16 if p == 0 else 2
        for kt in range(KT):
            scores_pair(p, kt)
            pump(budget)
            av_q.append((p, kt))
            if len(av_q) > LAG:
                pp, ii = av_q.popleft()
                av_pair(pp, ii)
                if ii == KT - 1 and pp != PAIRS - 1:
                    fillers.append(norm_gen(pp))
                pump(2)
    while av_q:
        pp, ii = av_q.popleft()
        av_pair(pp, ii)
    pump(10 ** 9)
    # tail: pair 7's normalization drains under the first two out_proj
    # chunks (k 0..4 only touch XA[0..4] = pairs <= 6); lag-2 pipeline over
    # two slot-pairs (psS x2) and (psS, psO).
    ng = norm_gen(PAIRS - 1)
    NT = QH // 128

    def pump_ng(n):
        for _ in range(n):
            try:
                next(ng)
            except StopIteration:
                break

    ps_of = {}
    pump_ng(3)  # sum-row copies, DRAM bounce, reciprocal queued
    ps_of[0] = out_proj_04(0, psS, "PSS")
    pump_ng(100)  # bcast matmuls + multiplies + XA scatter (exhaust ng)
    ps_of[1] = out_proj_04(1, psO_pool, "PSO")
    for t in range(2, NT):
        out_proj_5(t - 2, ps_of.pop(t - 2))
        if t % 2 == 0:
            ps_of[t] = out_proj_04(t, psS, "PSS")
        else:
            ps_of[t] = out_proj_04(t, psO_pool, "PSO")
    out_proj_5(NT - 2, ps_of.pop(NT - 2))
    out_proj_5(NT - 1, ps_of.pop(NT - 1))

    for pool in (outp, xap, rsp, ptp, psM, psO_pool, psS, persist):
        pool.release()


def _build():
    import concourse.mybir as mybir
    import concourse.tile as tile
    from concourse import bacc

    f32 = mybir.dt.float32
    bf16 = mybir.dt.bfloat16

    nc = bacc.Bacc("TRN2", target_bir_lowering=False, debug=False, num_devices=NCORES)
    nc.dram_tensor_handles = {}

    def decl(name, shape, dtype, is_out=False):
        h = nc.declare_dram_parameter(name, list(shape), dtype, isOutput=is_out)
        nc.dram_tensor_handles[name] = h
        return h

    decl("xT", [DIM, N], bf16)
    decl("wqT", [DIM, PAIRS, 128], bf16)
    decl("wkT", [DIM, PAIRS, 128], bf16)
    decl("wvT", [DIM, DIM], bf16)
    decl("woT", [DIM, DIM], bf16)
    decl("qb", [128, PAIRS], f32)
    decl("kb", [128, PAIRS], f32)
    # [partition, query-tile, dim]: host transposes back to [QH, DIM]
    decl("out", [128, QH // 128, DIM], bf16, is_out=True)

    with tile.TileContext(nc) as tc:
        _emit(tc, nc)
    nc.compile()
    return nc


def _host_prep(x, qkv_w, qkv_b, out_w, out_b):
    x = np.asarray(x, np.float32)
    qkv_w = np.asarray(qkv_w, np.float32)
    qkv_b = np.asarray(qkv_b, np.float32)
    out_w = np.asarray(out_w, np.float32)
    out_b = np.asarray(out_b, np.float32)

    wq, wk = qkv_w[0:DIM], qkv_w[DIM:2 * DIM]
    wv = qkv_w[2 * DIM:3 * DIM]

    def pack_pairs(w):  # w: [768(out), 768(in)] -> [768(in), 8, 128] padded
        wT = w.T
        out_arr = np.zeros((DIM, PAIRS, 128), np.float32)
        for j in range(PAIRS):
            out_arr[:, j, 0:DK] = wT[:, 96 * j:96 * j + DK]
            out_arr[:, j, 64:64 + DK] = wT[:, 96 * j + DK:96 * j + 96]
        return out_arr.astype(BF16)

    def pack_bias(bvec):  # [768] -> [128, 8] padded
        out_arr = np.zeros((128, PAIRS), np.float32)
        for j in range(PAIRS):
            out_arr[0:DK, j] = bvec[96 * j:96 * j + DK]
            out_arr[64:64 + DK, j] = bvec[96 * j + DK:96 * j + 96]
        return out_arr

    common = {
        "wqT": pack_pairs(wq),
        "wkT": pack_pairs(wk),
        "wvT": np.ascontiguousarray(wv.T).astype(BF16),
        "woT": np.ascontiguousarray(out_w.T).astype(BF16),
        "qb": pack_bias(qkv_b[0:DIM]),
        "kb": pack_bias(qkv_b[DIM:2 * DIM]),
    }
    xT_all = np.ascontiguousarray(x.transpose(0, 2, 1)).astype(BF16)  # [B, 768, N]
    in_maps = []
    for c in range(NCORES):
        b, qh = c // 2, c % 2
        mcore = dict(common)
        # core-local key order: own query-half first (softmax is invariant
        # to key permutation; makes the query block columns [0, QH) on every
        # core, so one SPMD program serves both pair members)
        mcore["xT"] = np.ascontiguousarray(np.concatenate(
            [xT_all[b][:, qh * QH:(qh + 1) * QH],
             xT_all[b][:, (1 - qh) * QH:(2 - qh) * QH]], axis=1))
        in_maps.append(mcore)
    return in_maps


def _run(in_maps, trace=False):
    global _compiled
    from concourse.bass_utils import run_bass_kernel_spmd

    if _compiled is None:
        _compiled = _build()
    return run_bass_kernel_spmd(_compiled, in_maps, list(range(NCORES)), trace=trace)


def kernel(x, qkv_w, qkv_b, out_w, out_b):
    in_maps = _host_prep(x, qkv_w, qkv_b, out_w, out_b)
    res = _run(in_maps, trace=False)
    # bias row (V-bias's out_proj image + output bias) added here in f32
    birow = (np.asarray(qkv_b, np.float32)[2 * DIM:] @ np.asarray(out_w, np.float32).T
             + np.asarray(out_b, np.float32))
    out = np.empty((B, N, DIM), np.float32)
    for c in range(NCORES):
        b, qh = c // 2, c % 2
        y = res.results[c]["out"].astype(np.float32)  # [128, QH//128, DIM]
        out[b, qh * QH:(qh + 1) * QH] = y.transpose(1, 0, 2).reshape(QH, DIM) + birow
    return out
